# revision 7
# baseline (speedup 1.0000x reference)
"""Trainium2 Bass kernel for gumbel-softmax attention.

Reference computation (all f32):
    scores = Q @ K.T / sqrt(64)            # [16384, 4096]
    p      = softmax(scores + g, axis=-1)  # g: fixed Gumbel noise, key 42
    out    = p @ V                         # [16384, 1024]
    return (out, p)

Strategy: shard genes (rows of Q) over 8 cores, 2048 genes/core; K, V
replicated.  Everything on-device is computed in TRANSPOSED layout
[latent, genes] so that the latent (contraction) dim lands on SBUF
partitions for both matmuls -- no on-device transposes needed:

    scoresT = KT_chunk.T @ QT              (PE, bf16)
    logitsT = scoresT + gT                 (DVE, fp16 noise, in-place PSUM)
    expT    = exp(logitsT)                 (ACT, -> bf16; no max-sub needed:
                                            logits <= ~25, exp fits f32)
    rowsumT = ones.T @ expT                (PE, [1, genes])
    out     = (expT.T @ V) * recip         (PE accumulate + DVE scale)
    pT      = expT * bcast(recipT)         (GPSIMD partition_broadcast + DVE)

The Gumbel noise is input-independent (fixed key) -> precomputed on host
with jax threefry (bit-exact vs reference) and streamed in as fp16.
Host assembles/unshards and transposes pT back to p.
"""

import os
import numpy as np

N_GENES, N_LATENT, D_K, N_CELLS = 16384, 4096, 64, 1024
N_CORES = 8
GPC = N_GENES // N_CORES      # genes per core = 2048
GB = 512                      # gene block width
NBLK = GPC // GB              # gene blocks per core = 4
NSB = GB // 128               # 128-gene sub-blocks per block = 4
NLT = N_LATENT // 128         # latent tiles = 32

_CACHE = {}


def _gumbel_noise():
    """Bit-exact reproduction of the reference's fixed Gumbel noise.

    Must run on the DEFAULT jax backend with default PRNG config: this
    environment uses `jax_default_prng_impl=rbg`, whose bits are
    backend-dependent, and the reference computes on the default device.
    """
    import jax
    import jax.numpy as jnp

    key = jax.random.key(42)
    u = jax.random.uniform(
        key, (N_GENES, N_LATENT), dtype=jnp.float32,
        minval=float(np.finfo(np.float32).tiny), maxval=1.0,
    )
    return np.asarray(-jnp.log(-jnp.log(u)))


def _build_bass():
    import contextlib
    import concourse.bacc as bacc
    import concourse.tile as tile
    from concourse import mybir
    from concourse.masks import make_identity

    f32 = mybir.dt.float32
    bf16 = mybir.dt.bfloat16
    f16 = mybir.dt.float16

    nc = bacc.Bacc("TRN2", target_bir_lowering=False, debug=False,
                   num_devices=N_CORES)

    qt = nc.dram_tensor("qt", [D_K, GPC], bf16, kind="ExternalInput").ap()
    kt = nc.dram_tensor("kt", [D_K, N_LATENT], bf16, kind="ExternalInput").ap()
    vp = nc.dram_tensor("vp", [N_LATENT, N_CELLS], bf16, kind="ExternalInput").ap()
    gt = nc.dram_tensor("gt", [N_LATENT, GPC], f16, kind="ExternalInput").ap()
    pt = nc.dram_tensor("pt", [N_LATENT, GPC], f32, kind="ExternalOutput").ap()
    outp = nc.dram_tensor("outp", [GPC, N_CELLS], f32, kind="ExternalOutput").ap()

    Exp = mybir.ActivationFunctionType.Exp
    Copy = mybir.ActivationFunctionType.Copy

    with tile.TileContext(nc) as tc, contextlib.ExitStack() as ctx:
        consts = ctx.enter_context(tc.tile_pool(name="consts", bufs=1))
        gpool = ctx.enter_context(tc.tile_pool(name="gpool", bufs=4))
        epool = ctx.enter_context(tc.tile_pool(name="epool", bufs=2))
        ptpool = ctx.enter_context(tc.tile_pool(name="ptpool", bufs=4))
        outpool = ctx.enter_context(tc.tile_pool(name="outpool", bufs=3))
        bcpool = ctx.enter_context(tc.tile_pool(name="bcpool", bufs=2))
        rows = ctx.enter_context(tc.tile_pool(name="rows", bufs=2))
        scpool = ctx.enter_context(tc.tile_pool(name="scpool", bufs=3, space="PSUM"))
        popool = ctx.enter_context(tc.tile_pool(name="popool", bufs=2, space="PSUM"))
        # rs and rT have disjoint lifetimes -> share one 1-bank slot
        rspool = ctx.enter_context(tc.tile_pool(name="rspool", bufs=1, space="PSUM"))

        ident = consts.tile([128, 128], f32)
        make_identity(nc, ident)
        ones_b = consts.tile([128, 1], bf16)
        nc.vector.memset(ones_b, 1.0)

        qt_s = consts.tile([D_K, GPC], bf16)
        nc.sync.dma_start(out=qt_s, in_=qt)
        kt_s = consts.tile([D_K, N_LATENT], bf16)
        nc.sync.dma_start(out=kt_s, in_=kt)
        vp_s = consts.tile([128, NLT, N_CELLS], bf16)
        vp_r = vp.rearrange("(t p) c -> p t c", p=128)

        # Per-block state threaded through the software pipeline.
        exps_t = [None] * NBLK
        rs_t = [None] * NBLK
        rbc_t = [None] * NBLK
        rcp_sb_t = [None] * NBLK

        def phase_a_steps(b):
            """Generator of per-latent-tile steps for block b (scores ->
            +noise -> exp -> rowsum-accumulate), to be interleaved with the
            previous block's PE-dense phase B."""
            exps = epool.tile([128, NLT, GB], bf16, name="exps")
            rs = rspool.tile([1, GB], f32, name="rs", tag="rs")
            exps_t[b] = exps
            rs_t[b] = rs
            for i in range(NLT):
                gti = gpool.tile([128, GB], f16, name="gti")
                nc.sync.dma_start(
                    out=gti, in_=gt[i * 128:(i + 1) * 128, b * GB:(b + 1) * GB])
                if b == 0:
                    # hide the V preload behind block 0's noise stream
                    nc.sync.dma_start(out=vp_s[:, i, :], in_=vp_r[:, i, :])
                sc = scpool.tile([128, GB], f32, name="sc")
                nc.tensor.matmul(
                    sc, lhsT=kt_s[:, i * 128:(i + 1) * 128],
                    rhs=qt_s[:, b * GB:(b + 1) * GB], start=True, stop=True)
                nc.vector.tensor_add(sc, sc, gti)
                nc.scalar.activation(out=exps[:, i, :], in_=sc, func=Exp)
                nc.tensor.matmul(rs, lhsT=ones_b, rhs=exps[:, i, :],
                                 start=(i == 0), stop=(i == NLT - 1))
                yield

        def emit_recips(b):
            """Rowsum -> reciprocal in both orientations (tiny)."""
            rs = rs_t[b]
            rsr = rows.tile([1, GB], f32, name="rsr")
            nc.scalar.copy(out=rsr, in_=rs)
            rcp = rows.tile([1, GB], f32, name="rcp")
            nc.vector.reciprocal(rcp, rsr)
            rbc = bcpool.tile([128, GB], f32, name="rbc")
            nc.gpsimd.partition_broadcast(rbc, rcp)
            rT = rspool.tile([128, NSB], f32, name="rT", tag="rs")
            for s in range(NSB):
                nc.tensor.transpose(
                    rT[:, s:s + 1], rsr[0:1, s * 128:(s + 1) * 128],
                    ident[0:1, 0:1])
            rts = rows.tile([128, NSB], f32, name="rts")
            nc.scalar.copy(out=rts, in_=rT)
            rcp_sb = rows.tile([128, NSB], f32, name="rcp_sb")
            nc.vector.reciprocal(rcp_sb, rts)
            rbc_t[b] = rbc
            rcp_sb_t[b] = rcp_sb

        def emit_phase_c(b):
            """pT = expT * bcast(recipT) on GpSimd; runs alongside B's PE
            work (emitted first: its inputs are ready before B starts)."""
            exps, rbc = exps_t[b], rbc_t[b]
            for i in range(NLT):
                ptile = ptpool.tile([128, GB], f32, name="ptile")
                nc.gpsimd.tensor_mul(ptile, exps[:, i, :], rbc)
                nc.sync.dma_start(
                    out=pt[i * 128:(i + 1) * 128, b * GB:(b + 1) * GB],
                    in_=ptile)

        def emit_phase_b(b, a_next):
            """out = (expT.T @ V) * recip per 128-gene sub-block, with the
            next block's phase-A steps interleaved (1 per 4 i-steps) so
            DVE/ACT chew block b+1 while PE runs block b's matmuls."""
            exps, rcp_sb = exps_t[b], rcp_sb_t[b]
            for s in range(NSB):
                po = popool.tile([128, N_CELLS], f32, name="po")
                for i in range(NLT):
                    lhs = exps[:, i, s * 128:(s + 1) * 128]
                    nc.tensor.matmul(po[:, 0:512], lhsT=lhs,
                                     rhs=vp_s[:, i, 0:512],
                                     start=(i == 0), stop=(i == NLT - 1))
                    nc.tensor.matmul(po[:, 512:1024], lhsT=lhs,
                                     rhs=vp_s[:, i, 512:1024],
                                     start=(i == 0), stop=(i == NLT - 1))
                    if a_next is not None and i % 4 == 3:
                        next(a_next, None)
                outn = outpool.tile([128, N_CELLS], f32, name="outn")
                nc.scalar.activation(out=outn, in_=po, func=Copy,
                                     scale=rcp_sb[:, s:s + 1])
                g0 = (b * NSB + s) * 128
                nc.sync.dma_start(out=outp[g0:g0 + 128, :], in_=outn)
            if a_next is not None:
                for _ in a_next:
                    pass

        # Software pipeline: A(0); then per block: recips, C, B+A(next).
        for _ in phase_a_steps(0):
            pass
        for b in range(NBLK):
            emit_recips(b)
            emit_phase_c(b)
            a_next = phase_a_steps(b + 1) if b + 1 < NBLK else None
            emit_phase_b(b, a_next)

    nc.compile()
    return nc


def _prep_inputs(query, key, value):
    import ml_dtypes

    bf16 = ml_dtypes.bfloat16
    g = _gumbel_noise()
    qt_full = (query.astype(np.float32).T / 8.0).astype(bf16)   # [64, 16384]
    kt = np.ascontiguousarray(key.astype(np.float32).T).astype(bf16)
    vp = value.astype(bf16)
    gt_full = g.T.astype(np.float16)                             # [4096, 16384]

    in_maps = []
    for c in range(N_CORES):
        sl = slice(c * GPC, (c + 1) * GPC)
        in_maps.append({
            "qt": np.ascontiguousarray(qt_full[:, sl]),
            "kt": kt,
            "vp": vp,
            "gt": np.ascontiguousarray(gt_full[:, sl]),
        })
    return in_maps


LAST_RESULT = None


def kernel(query, key, value):
    global LAST_RESULT
    from concourse.bass_utils import run_bass_kernel_spmd

    if "nc" not in _CACHE:
        _CACHE["nc"] = _build_bass()
    nc = _CACHE["nc"]

    in_maps = _prep_inputs(np.asarray(query), np.asarray(key), np.asarray(value))
    res = run_bass_kernel_spmd(
        nc, in_maps, core_ids=list(range(N_CORES)),
        trace=bool(int(os.environ.get("KERNEL_TRACE", "0"))))
    LAST_RESULT = res

    out = np.empty((N_GENES, N_CELLS), np.float32)
    p = np.empty((N_GENES, N_LATENT), np.float32)
    for c in range(N_CORES):
        sl = slice(c * GPC, (c + 1) * GPC)
        out[sl] = res.results[c]["outp"]
        p[sl] = res.results[c]["pt"].T
    return out, p


# revision 8
# speedup vs baseline: 1.1421x; 1.1421x over previous
"""Trainium2 Bass kernel for gumbel-softmax attention.

Reference computation (all f32):
    scores = Q @ K.T / sqrt(64)            # [16384, 4096]
    p      = softmax(scores + g, axis=-1)  # g: fixed Gumbel noise, key 42
    out    = p @ V                         # [16384, 1024]
    return (out, p)

Strategy: shard genes (rows of Q) over 8 cores, 2048 genes/core; K, V
replicated.  Everything on-device is computed in TRANSPOSED layout
[latent, genes] so the latent (contraction) dim lands on SBUF partitions
for both matmuls -- no on-device transposes needed:

    scoresT = KT_chunk.T @ QT              (PE, bf16)
    logitsT = scoresT + gT                 (DVE, fp16 noise, in-place PSUM)
    expT    = exp(logitsT)                 (ACT, -> bf16; no max-sub needed:
                                            logits <= ~25, exp fits f32)
    rowsumT = ones.T @ expT                (PE, [1, genes] accumulated)
    out_un  = expT.T @ V                   (PE accumulate, copied out via ACT)

The device ships expT (bf16), out_un and rowsums; the HOST applies the
softmax normalization (a per-gene f32 scale -- bit-identical math to
doing it on-device, but halves the p-output traffic and removes all
cross-engine normalization dependencies from the device pipeline).

The Gumbel noise is input-independent (fixed key) -> precomputed on host
with the same jax call the reference makes (this environment uses the
rbg PRNG whose bits are backend-dependent -- must run on the default
backend) and streamed in as fp16.
"""

import os
import numpy as np

N_GENES, N_LATENT, D_K, N_CELLS = 16384, 4096, 64, 1024
N_CORES = 8
GPC = N_GENES // N_CORES      # genes per core = 2048
GB = 512                      # gene block width
NBLK = GPC // GB              # gene blocks per core = 4
NSB = GB // 128               # 128-gene sub-blocks per block = 4
NLT = N_LATENT // 128         # latent tiles = 32

_CACHE = {}


def _gumbel_noise():
    """Bit-exact reproduction of the reference's fixed Gumbel noise.

    Must run on the DEFAULT jax backend with default PRNG config: this
    environment uses `jax_default_prng_impl=rbg`, whose bits are
    backend-dependent, and the reference computes on the default device.
    """
    import jax
    import jax.numpy as jnp

    key = jax.random.key(42)
    u = jax.random.uniform(
        key, (N_GENES, N_LATENT), dtype=jnp.float32,
        minval=float(np.finfo(np.float32).tiny), maxval=1.0,
    )
    return np.asarray(-jnp.log(-jnp.log(u)))


def _build_bass():
    import contextlib
    import concourse.bacc as bacc
    import concourse.tile as tile
    from concourse import mybir

    f32 = mybir.dt.float32
    bf16 = mybir.dt.bfloat16
    f16 = mybir.dt.float16

    nc = bacc.Bacc("TRN2", target_bir_lowering=False, debug=False,
                   num_devices=N_CORES)

    qt = nc.dram_tensor("qt", [D_K, GPC], bf16, kind="ExternalInput").ap()
    kt = nc.dram_tensor("kt", [D_K, N_LATENT], bf16, kind="ExternalInput").ap()
    vp = nc.dram_tensor("vp", [N_LATENT, N_CELLS], bf16, kind="ExternalInput").ap()
    gt = nc.dram_tensor("gt", [N_LATENT, GPC], f16, kind="ExternalInput").ap()
    pt = nc.dram_tensor("pt", [N_LATENT, GPC], bf16, kind="ExternalOutput").ap()
    outp = nc.dram_tensor("outp", [GPC, N_CELLS], f32, kind="ExternalOutput").ap()
    rsums = nc.dram_tensor("rsums", [NBLK, GB], f32, kind="ExternalOutput").ap()

    Exp = mybir.ActivationFunctionType.Exp

    with tile.TileContext(nc) as tc, contextlib.ExitStack() as ctx:
        consts = ctx.enter_context(tc.tile_pool(name="consts", bufs=1))
        gpool = ctx.enter_context(tc.tile_pool(name="gpool", bufs=6))
        epool = ctx.enter_context(tc.tile_pool(name="epool", bufs=2))
        outpool = ctx.enter_context(tc.tile_pool(name="outpool", bufs=3))
        rows = ctx.enter_context(tc.tile_pool(name="rows", bufs=2))
        scpool = ctx.enter_context(tc.tile_pool(name="scpool", bufs=3, space="PSUM"))
        popool = ctx.enter_context(tc.tile_pool(name="popool", bufs=2, space="PSUM"))
        rspool = ctx.enter_context(tc.tile_pool(name="rspool", bufs=1, space="PSUM"))

        ones_b = consts.tile([128, 1], bf16)
        nc.vector.memset(ones_b, 1.0)

        qt_s = consts.tile([D_K, GPC], bf16)
        nc.sync.dma_start(out=qt_s, in_=qt)
        kt_s = consts.tile([D_K, N_LATENT], bf16)
        nc.sync.dma_start(out=kt_s, in_=kt)
        vp_s = consts.tile([128, NLT, N_CELLS], bf16)
        vp_r = vp.rearrange("(t p) c -> p t c", p=128)

        exps_t = [None] * NBLK
        rs_t = [None] * NBLK

        def phase_a_steps(b):
            """Per-latent-tile steps for block b: noise DMA, scores MM,
            +noise (DVE), exp (ACT), expT DMA out, rowsum-accumulate MM
            (lagged 2 tiles so PE doesn't convoy on ACT)."""
            exps = epool.tile([128, NLT, GB], bf16, name="exps")
            rs = rspool.tile([1, GB], f32, name="rs", tag="rs")
            exps_t[b] = exps
            rs_t[b] = rs
            for i in range(NLT):
                gti = gpool.tile([128, GB], f16, name="gti")
                nc.sync.dma_start(
                    out=gti, in_=gt[i * 128:(i + 1) * 128, b * GB:(b + 1) * GB])
                if b == 0:
                    # hide the V preload behind block 0's noise stream
                    nc.sync.dma_start(out=vp_s[:, i, :], in_=vp_r[:, i, :])
                sc = scpool.tile([128, GB], f32, name="sc")
                nc.tensor.matmul(
                    sc, lhsT=kt_s[:, i * 128:(i + 1) * 128],
                    rhs=qt_s[:, b * GB:(b + 1) * GB], start=True, stop=True)
                nc.vector.tensor_add(sc, sc, gti)
                nc.scalar.activation(out=exps[:, i, :], in_=sc, func=Exp)
                nc.sync.dma_start(
                    out=pt[i * 128:(i + 1) * 128, b * GB:(b + 1) * GB],
                    in_=exps[:, i, :])
                if i >= 2:
                    nc.tensor.matmul(rs, lhsT=ones_b, rhs=exps[:, i - 2, :],
                                     start=(i == 2), stop=False)
                yield
            for i in (NLT - 2, NLT - 1):
                nc.tensor.matmul(rs, lhsT=ones_b, rhs=exps[:, i, :],
                                 start=False, stop=(i == NLT - 1))

        def emit_rowsum_out(b):
            rsr = rows.tile([1, GB], f32, name="rsr")
            nc.scalar.copy(out=rsr, in_=rs_t[b])
            nc.sync.dma_start(out=rsums[b:b + 1, :], in_=rsr)

        def emit_phase_b(b, a_next):
            """out_un = expT.T @ V per 128-gene sub-block, with the next
            block's phase-A steps interleaved (1 per 4 i-steps) so DVE/ACT
            chew block b+1 while PE runs block b's matmuls."""
            exps = exps_t[b]
            for s in range(NSB):
                po = popool.tile([128, N_CELLS], f32, name="po")
                for i in range(NLT):
                    lhs = exps[:, i, s * 128:(s + 1) * 128]
                    nc.tensor.matmul(po[:, 0:512], lhsT=lhs,
                                     rhs=vp_s[:, i, 0:512],
                                     start=(i == 0), stop=(i == NLT - 1))
                    nc.tensor.matmul(po[:, 512:1024], lhsT=lhs,
                                     rhs=vp_s[:, i, 512:1024],
                                     start=(i == 0), stop=(i == NLT - 1))
                    if a_next is not None and i % 4 == 3:
                        next(a_next, None)
                outn = outpool.tile([128, N_CELLS], f32, name="outn")
                nc.scalar.copy(out=outn, in_=po)
                g0 = (b * NSB + s) * 128
                nc.sync.dma_start(out=outp[g0:g0 + 128, :], in_=outn)
            if a_next is not None:
                for _ in a_next:
                    pass

        # Software pipeline: A(0); then per block: rowsums out, B + A(next).
        for _ in phase_a_steps(0):
            pass
        for b in range(NBLK):
            emit_rowsum_out(b)
            a_next = phase_a_steps(b + 1) if b + 1 < NBLK else None
            emit_phase_b(b, a_next)

    nc.compile()
    return nc


def _prep_inputs(query, key, value):
    import ml_dtypes

    bf16 = ml_dtypes.bfloat16
    g = _gumbel_noise()
    qt_full = (query.astype(np.float32).T / 8.0).astype(bf16)   # [64, 16384]
    kt = np.ascontiguousarray(key.astype(np.float32).T).astype(bf16)
    vp = value.astype(bf16)
    gt_full = g.T.astype(np.float16)                             # [4096, 16384]

    in_maps = []
    for c in range(N_CORES):
        sl = slice(c * GPC, (c + 1) * GPC)
        in_maps.append({
            "qt": np.ascontiguousarray(qt_full[:, sl]),
            "kt": kt,
            "vp": vp,
            "gt": np.ascontiguousarray(gt_full[:, sl]),
        })
    return in_maps


LAST_RESULT = None


def kernel(query, key, value):
    global LAST_RESULT
    from concourse.bass_utils import run_bass_kernel_spmd

    if "nc" not in _CACHE:
        _CACHE["nc"] = _build_bass()
    nc = _CACHE["nc"]

    in_maps = _prep_inputs(np.asarray(query), np.asarray(key), np.asarray(value))
    res = run_bass_kernel_spmd(
        nc, in_maps, core_ids=list(range(N_CORES)),
        trace=bool(int(os.environ.get("KERNEL_TRACE", "0"))))
    LAST_RESULT = res

    out = np.empty((N_GENES, N_CELLS), np.float32)
    p = np.empty((N_GENES, N_LATENT), np.float32)
    for c in range(N_CORES):
        sl = slice(c * GPC, (c + 1) * GPC)
        r = res.results[c]
        recip = (1.0 / r["rsums"].reshape(GPC)).astype(np.float32)
        out[sl] = r["outp"] * recip[:, None]
        # p[g, l] = expT[l, g] * recip[g]
        p[sl] = (r["pt"].astype(np.float32) * recip[None, :]).T
    return out, p


# revision 13
# speedup vs baseline: 1.3623x; 1.1928x over previous
"""Trainium2 Bass kernel for gumbel-softmax attention.

Reference computation (all f32):
    scores = Q @ K.T / sqrt(64)            # [16384, 4096]
    p      = softmax(scores + g, axis=-1)  # g: fixed Gumbel noise, key 42
    out    = p @ V                         # [16384, 1024]
    return (out, p)

Strategy: shard genes (rows of Q) over 8 cores, 2048 genes/core; K, V
replicated.  Everything on-device is computed in TRANSPOSED layout
[latent, genes] so the latent (contraction) dim lands on SBUF partitions
for both matmuls -- no on-device transposes needed:

    scoresT = KT_chunk.T @ QT              (PE, bf16)
    logitsT = scoresT + gT                 (DVE, fp16 noise, in-place PSUM)
    expT    = exp(logitsT)                 (ACT, -> bf16; no max-sub needed:
                                            logits <= ~25, exp fits f32)
    rowsumT = ones.T @ expT                (PE, [1, genes] accumulated)
    out_un  = expT.T @ V                   (PE accumulate, copied out via ACT)

The device ships expT (bf16), out_un and rowsums; the HOST applies the
softmax normalization (a per-gene f32 scale -- bit-identical math to
doing it on-device, but halves the p-output traffic and removes all
cross-engine normalization dependencies from the device pipeline).

The Gumbel noise is input-independent (fixed key) -> precomputed on host
with the same jax call the reference makes (this environment uses the
rbg PRNG whose bits are backend-dependent -- must run on the default
backend) and streamed in as fp16.
"""

import os
import numpy as np

N_GENES, N_LATENT, D_K, N_CELLS = 16384, 4096, 64, 1024
N_CORES = 8
GPC = N_GENES // N_CORES      # genes per core = 2048
GB = 512                      # gene block width
NBLK = GPC // GB              # gene blocks per core = 4
NSB = GB // 128               # 128-gene sub-blocks per block = 4
NLT = N_LATENT // 128         # latent tiles = 32

_CACHE = {}


def _gumbel_noise():
    """Bit-exact reproduction of the reference's fixed Gumbel noise.

    Must run on the DEFAULT jax backend with default PRNG config: this
    environment uses `jax_default_prng_impl=rbg`, whose bits are
    backend-dependent, and the reference computes on the default device.
    """
    import jax
    import jax.numpy as jnp

    key = jax.random.key(42)
    u = jax.random.uniform(
        key, (N_GENES, N_LATENT), dtype=jnp.float32,
        minval=float(np.finfo(np.float32).tiny), maxval=1.0,
    )
    return np.asarray(-jnp.log(-jnp.log(u)))


def _build_bass():
    import contextlib
    import concourse.bacc as bacc
    import concourse.tile as tile
    from concourse import mybir

    f32 = mybir.dt.float32
    bf16 = mybir.dt.bfloat16
    f16 = mybir.dt.float16

    nc = bacc.Bacc("TRN2", target_bir_lowering=False, debug=False,
                   num_devices=N_CORES)

    qt = nc.dram_tensor("qt", [D_K, GPC], bf16, kind="ExternalInput").ap()
    kt = nc.dram_tensor("kt", [D_K, N_LATENT], bf16, kind="ExternalInput").ap()
    vp = nc.dram_tensor("vp", [N_LATENT, N_CELLS], bf16, kind="ExternalInput").ap()
    gt = nc.dram_tensor("gt", [N_LATENT, GPC], f16, kind="ExternalInput").ap()
    pt = nc.dram_tensor("pt", [N_LATENT, GPC], bf16, kind="ExternalOutput").ap()
    outp = nc.dram_tensor("outp", [GPC, N_CELLS], f32, kind="ExternalOutput").ap()

    Exp = mybir.ActivationFunctionType.Exp

    with tile.TileContext(nc) as tc, contextlib.ExitStack() as ctx:
        consts = ctx.enter_context(tc.tile_pool(name="consts", bufs=1))
        gpool = ctx.enter_context(tc.tile_pool(name="gpool", bufs=6))
        epool = ctx.enter_context(tc.tile_pool(name="epool", bufs=2))
        outpool = ctx.enter_context(tc.tile_pool(name="outpool", bufs=3))
        scpool = ctx.enter_context(tc.tile_pool(name="scpool", bufs=4, space="PSUM"))
        popool = ctx.enter_context(tc.tile_pool(name="popool", bufs=2, space="PSUM"))

        qt_s = consts.tile([D_K, GPC], bf16)
        nc.sync.dma_start(out=qt_s, in_=qt)
        kt_s = consts.tile([D_K, N_LATENT], bf16)
        nc.sync.dma_start(out=kt_s, in_=kt)
        vp_s = consts.tile([128, NLT, N_CELLS], bf16)
        vp_r = vp.rearrange("(t p) c -> p t c", p=128)

        exps_t = [None] * NBLK

        def phase_a_steps(b):
            """Per-latent-tile steps for block b: noise DMA, scores MM,
            +noise (DVE), exp (ACT), expT DMA out (on GpSimd queues --
            Sync is the busier DMA issuer)."""
            exps = epool.tile([128, NLT, GB], bf16, name="exps")
            exps_t[b] = exps
            for i in range(NLT):
                gti = gpool.tile([128, GB], f16, name="gti")
                nc.sync.dma_start(
                    out=gti, in_=gt[i * 128:(i + 1) * 128, b * GB:(b + 1) * GB])
                if b == 0:
                    # hide the V preload behind block 0's noise stream
                    nc.sync.dma_start(out=vp_s[:, i, :], in_=vp_r[:, i, :])
                sc = scpool.tile([128, GB], f32, name="sc")
                nc.tensor.matmul(
                    sc, lhsT=kt_s[:, i * 128:(i + 1) * 128],
                    rhs=qt_s[:, b * GB:(b + 1) * GB], start=True, stop=True)
                nc.vector.tensor_add(sc, sc, gti)
                nc.scalar.activation(out=exps[:, i, :], in_=sc, func=Exp)
                nc.gpsimd.dma_start(
                    out=pt[i * 128:(i + 1) * 128, b * GB:(b + 1) * GB],
                    in_=exps[:, i, :])
                yield

        def emit_phase_b(b, a_next):
            """out_un = expT.T @ V per 128-gene sub-block, with the next
            block's phase-A steps interleaved (1 per 4 i-steps) so DVE/ACT
            chew block b+1 while PE runs block b's matmuls."""
            exps = exps_t[b]
            for s in range(NSB):
                po = popool.tile([128, N_CELLS], f32, name="po")
                for i in range(NLT):
                    lhs = exps[:, i, s * 128:(s + 1) * 128]
                    nc.tensor.matmul(po[:, 0:512], lhsT=lhs,
                                     rhs=vp_s[:, i, 0:512],
                                     start=(i == 0), stop=(i == NLT - 1))
                    nc.tensor.matmul(po[:, 512:1024], lhsT=lhs,
                                     rhs=vp_s[:, i, 512:1024],
                                     start=(i == 0), stop=(i == NLT - 1))
                    if a_next is not None and i % 4 == 3:
                        next(a_next, None)
                outn = outpool.tile([128, N_CELLS], f32, name="outn")
                nc.scalar.copy(out=outn, in_=po)
                g0 = (b * NSB + s) * 128
                nc.sync.dma_start(out=outp[g0:g0 + 128, :], in_=outn)
            if a_next is not None:
                for _ in a_next:
                    pass

        # Software pipeline: A(0); then per block: B + A(next) interleaved.
        for _ in phase_a_steps(0):
            pass
        for b in range(NBLK):
            a_next = phase_a_steps(b + 1) if b + 1 < NBLK else None
            emit_phase_b(b, a_next)

    nc.compile()
    return nc


def _prep_inputs(query, key, value):
    import ml_dtypes

    bf16 = ml_dtypes.bfloat16
    g = _gumbel_noise()
    qt_full = (query.astype(np.float32).T / 8.0).astype(bf16)   # [64, 16384]
    kt = np.ascontiguousarray(key.astype(np.float32).T).astype(bf16)
    vp = value.astype(bf16)
    gt_full = g.T.astype(np.float16)                             # [4096, 16384]

    in_maps = []
    for c in range(N_CORES):
        sl = slice(c * GPC, (c + 1) * GPC)
        in_maps.append({
            "qt": np.ascontiguousarray(qt_full[:, sl]),
            "kt": kt,
            "vp": vp,
            "gt": np.ascontiguousarray(gt_full[:, sl]),
        })
    return in_maps


LAST_RESULT = None


def kernel(query, key, value):
    global LAST_RESULT
    from concourse.bass_utils import run_bass_kernel_spmd

    if "nc" not in _CACHE:
        _CACHE["nc"] = _build_bass()
    nc = _CACHE["nc"]

    in_maps = _prep_inputs(np.asarray(query), np.asarray(key), np.asarray(value))
    res = run_bass_kernel_spmd(
        nc, in_maps, core_ids=list(range(N_CORES)),
        trace=bool(int(os.environ.get("KERNEL_TRACE", "0"))))
    LAST_RESULT = res

    out = np.empty((N_GENES, N_CELLS), np.float32)
    p = np.empty((N_GENES, N_LATENT), np.float32)
    for c in range(N_CORES):
        sl = slice(c * GPC, (c + 1) * GPC)
        r = res.results[c]
        ptf = r["pt"].astype(np.float32)          # [latent, genes]
        recip = 1.0 / ptf.sum(axis=0)             # softmax denominators
        out[sl] = r["outp"] * recip[:, None]
        # p[g, l] = expT[l, g] * recip[g]
        p[sl] = (ptf * recip[None, :]).T
    return out, p


# revision 19
# speedup vs baseline: 1.4718x; 1.0804x over previous
"""Trainium2 Bass kernel for gumbel-softmax attention.

Reference computation (all f32):
    scores = Q @ K.T / sqrt(64)            # [16384, 4096]
    p      = softmax(scores + g, axis=-1)  # g: fixed Gumbel noise, key 42
    out    = p @ V                         # [16384, 1024]
    return (out, p)

Strategy: shard genes (rows of Q) over 8 cores, 2048 genes/core; K, V
replicated.  Everything on-device is computed in TRANSPOSED layout
[latent, genes] so the latent (contraction) dim lands on SBUF partitions
for both matmuls -- no on-device transposes needed:

    scoresT = KT_chunk.T @ QT              (PE, bf16)
    logitsT = scoresT + gT                 (DVE, fp16 noise, in-place PSUM)
    expT    = exp(logitsT)                 (ACT, -> bf16; no max-sub needed:
                                            logits <= ~25, exp fits f32)
    rowsumT = ones.T @ expT                (PE, [1, genes] accumulated)
    out_un  = expT.T @ V                   (PE accumulate, copied out via ACT)

The device ships expT (bf16), out_un and rowsums; the HOST applies the
softmax normalization (a per-gene f32 scale -- bit-identical math to
doing it on-device, but halves the p-output traffic and removes all
cross-engine normalization dependencies from the device pipeline).

The Gumbel noise is input-independent (fixed key) -> precomputed on host
with the same jax call the reference makes (this environment uses the
rbg PRNG whose bits are backend-dependent -- must run on the default
backend) and streamed in as fp16.
"""

import os
import numpy as np

N_GENES, N_LATENT, D_K, N_CELLS = 16384, 4096, 64, 1024
N_CORES = 8
GPC = N_GENES // N_CORES      # genes per core = 2048
GB = 512                      # gene block width
NBLK = GPC // GB              # gene blocks per core = 4
NSB = GB // 128               # 128-gene sub-blocks per block = 4
NLT = N_LATENT // 128         # latent tiles = 32

_CACHE = {}


def _gumbel_noise():
    """Bit-exact reproduction of the reference's fixed Gumbel noise.

    Must run on the DEFAULT jax backend with default PRNG config: this
    environment uses `jax_default_prng_impl=rbg`, whose bits are
    backend-dependent, and the reference computes on the default device.
    """
    import jax
    import jax.numpy as jnp

    key = jax.random.key(42)
    u = jax.random.uniform(
        key, (N_GENES, N_LATENT), dtype=jnp.float32,
        minval=float(np.finfo(np.float32).tiny), maxval=1.0,
    )
    return np.asarray(-jnp.log(-jnp.log(u)))


def _build_bass():
    import contextlib
    import concourse.bacc as bacc
    import concourse.tile as tile
    from concourse import mybir

    f32 = mybir.dt.float32
    bf16 = mybir.dt.bfloat16
    f16 = mybir.dt.float16

    nc = bacc.Bacc("TRN2", target_bir_lowering=False, debug=False,
                   num_devices=N_CORES)

    # qt/kt ship with rows duplicated into partitions 64..127 so two K=64
    # scores matmuls can run concurrently in the PE array's two row-halves.
    qt = nc.dram_tensor("qt", [2 * D_K, GPC], bf16, kind="ExternalInput").ap()
    kt = nc.dram_tensor("kt", [2 * D_K, N_LATENT], bf16, kind="ExternalInput").ap()
    vp = nc.dram_tensor("vp", [N_LATENT, N_CELLS], bf16, kind="ExternalInput").ap()
    gt = nc.dram_tensor("gt", [N_LATENT, GPC], f16, kind="ExternalInput").ap()
    pt = nc.dram_tensor("pt", [N_LATENT, GPC], bf16, kind="ExternalOutput").ap()
    outp = nc.dram_tensor("outp", [GPC, N_CELLS], f32, kind="ExternalOutput").ap()

    Exp = mybir.ActivationFunctionType.Exp

    with tile.TileContext(nc) as tc, contextlib.ExitStack() as ctx:
        consts = ctx.enter_context(tc.tile_pool(name="consts", bufs=1))
        gpool = ctx.enter_context(tc.tile_pool(name="gpool", bufs=6))
        epool = ctx.enter_context(tc.tile_pool(name="epool", bufs=2))
        outpool = ctx.enter_context(tc.tile_pool(name="outpool", bufs=3))
        scpool = ctx.enter_context(tc.tile_pool(name="scpool", bufs=2, space="PSUM"))
        popool = ctx.enter_context(tc.tile_pool(name="popool", bufs=2, space="PSUM"))

        qt_s = consts.tile([2 * D_K, GPC], bf16)
        for j in range(2):
            nc.sync.dma_start(out=qt_s[:, j * 1024:(j + 1) * 1024],
                              in_=qt[:, j * 1024:(j + 1) * 1024])
        kt_s = consts.tile([2 * D_K, N_LATENT], bf16)
        for j in range(4):
            nc.sync.dma_start(out=kt_s[:, j * 1024:(j + 1) * 1024],
                              in_=kt[:, j * 1024:(j + 1) * 1024])
        vp_s = consts.tile([128, NLT, N_CELLS], bf16)
        vp_r = vp.rearrange("(t p) c -> p t c", p=128)

        exps_t = [None] * NBLK
        PREF = 2  # gt prefetch distance, in packed (2-tile) steps

        def phase_a_steps(b):
            """Packed steps for block b, two latent tiles at a time:
            noise DMA (prefetched), 2 concurrent scores MMs (row-halves
            of the PE array), +noise (DVE), exp (ACT), expT DMA out (on
            GpSimd queues -- Sync is the busier DMA issuer)."""
            exps = epool.tile([128, NLT, GB], bf16, name="exps")
            exps_t[b] = exps
            gtiles = {}

            def fetch(k):
                if k >= NLT // 2:
                    return
                gti = gpool.tile([128, 2 * GB], f16, name="gti")
                i = 2 * k
                nc.sync.dma_start(
                    out=gti[:, 0:GB],
                    in_=gt[i * 128:(i + 1) * 128, b * GB:(b + 1) * GB])
                nc.sync.dma_start(
                    out=gti[:, GB:2 * GB],
                    in_=gt[(i + 1) * 128:(i + 2) * 128, b * GB:(b + 1) * GB])
                gtiles[k] = gti

            for k in range(PREF):
                fetch(k)
            for k in range(NLT // 2):
                i = 2 * k
                fetch(k + PREF)
                if b == 0:
                    # hide the V preload behind block 0's noise stream
                    nc.sync.dma_start(out=vp_s[:, i, :], in_=vp_r[:, i, :])
                    nc.sync.dma_start(out=vp_s[:, i + 1, :], in_=vp_r[:, i + 1, :])
                gti = gtiles.pop(k)
                sc = scpool.tile([128, 2 * GB], f32, name="sc")
                nc.tensor.matmul(
                    sc[:, 0:GB], lhsT=kt_s[0:D_K, i * 128:(i + 1) * 128],
                    rhs=qt_s[0:D_K, b * GB:(b + 1) * GB], start=True, stop=True)
                nc.tensor.matmul(
                    sc[:, GB:2 * GB],
                    lhsT=kt_s[D_K:2 * D_K, (i + 1) * 128:(i + 2) * 128],
                    rhs=qt_s[D_K:2 * D_K, b * GB:(b + 1) * GB],
                    start=True, stop=True)
                nc.vector.tensor_add(sc, sc, gti)
                nc.scalar.activation(out=exps[:, i:i + 2, :], in_=sc, func=Exp)
                pt_view = pt[i * 128:(i + 2) * 128, b * GB:(b + 1) * GB] \
                    .rearrange("(t p) g -> p t g", p=128)
                nc.gpsimd.dma_start(out=pt_view, in_=exps[:, i:i + 2, :])
                yield

        def emit_phase_b(b, a_next):
            """out_un = expT.T @ V per 128-gene sub-block, with the next
            block's phase-A steps interleaved (1 per 4 i-steps) so DVE/ACT
            chew block b+1 while PE runs block b's matmuls."""
            exps = exps_t[b]
            for s in range(NSB):
                po = popool.tile([128, N_CELLS], f32, name="po")
                for i in range(NLT):
                    lhs = exps[:, i, s * 128:(s + 1) * 128]
                    nc.tensor.matmul(po[:, 0:512], lhsT=lhs,
                                     rhs=vp_s[:, i, 0:512],
                                     start=(i == 0), stop=(i == NLT - 1))
                    nc.tensor.matmul(po[:, 512:1024], lhsT=lhs,
                                     rhs=vp_s[:, i, 512:1024],
                                     start=(i == 0), stop=(i == NLT - 1))
                    if a_next is not None and i % 8 == 3:
                        next(a_next, None)
                outn = outpool.tile([128, N_CELLS], f32, name="outn")
                nc.scalar.copy(out=outn, in_=po)
                g0 = (b * NSB + s) * 128
                nc.sync.dma_start(out=outp[g0:g0 + 128, :], in_=outn)
            if a_next is not None:
                for _ in a_next:
                    pass

        # Software pipeline: A(0); then per block: B + A(next) interleaved.
        for _ in phase_a_steps(0):
            pass
        for b in range(NBLK):
            a_next = phase_a_steps(b + 1) if b + 1 < NBLK else None
            emit_phase_b(b, a_next)

    nc.compile()
    return nc


def _prep_inputs(query, key, value):
    import ml_dtypes

    bf16 = ml_dtypes.bfloat16
    g = _gumbel_noise()
    qt_full = (query.astype(np.float32).T / 8.0).astype(bf16)   # [64, 16384]
    qt_full = np.vstack([qt_full, qt_full])                      # [128, 16384]
    kt1 = np.ascontiguousarray(key.astype(np.float32).T).astype(bf16)
    kt = np.vstack([kt1, kt1])                                   # [128, 4096]
    vp = value.astype(bf16)
    gt_full = g.T.astype(np.float16)                             # [4096, 16384]

    in_maps = []
    for c in range(N_CORES):
        sl = slice(c * GPC, (c + 1) * GPC)
        in_maps.append({
            "qt": np.ascontiguousarray(qt_full[:, sl]),
            "kt": kt,
            "vp": vp,
            "gt": np.ascontiguousarray(gt_full[:, sl]),
        })
    return in_maps


LAST_RESULT = None


def kernel(query, key, value):
    global LAST_RESULT
    from concourse.bass_utils import run_bass_kernel_spmd

    if "nc" not in _CACHE:
        _CACHE["nc"] = _build_bass()
    nc = _CACHE["nc"]

    in_maps = _prep_inputs(np.asarray(query), np.asarray(key), np.asarray(value))
    res = run_bass_kernel_spmd(
        nc, in_maps, core_ids=list(range(N_CORES)),
        trace=bool(int(os.environ.get("KERNEL_TRACE", "0"))))
    LAST_RESULT = res

    out = np.empty((N_GENES, N_CELLS), np.float32)
    p = np.empty((N_GENES, N_LATENT), np.float32)
    for c in range(N_CORES):
        sl = slice(c * GPC, (c + 1) * GPC)
        r = res.results[c]
        ptf = r["pt"].astype(np.float32)          # [latent, genes]
        recip = 1.0 / ptf.sum(axis=0)             # softmax denominators
        out[sl] = r["outp"] * recip[:, None]
        # p[g, l] = expT[l, g] * recip[g]
        p[sl] = (ptf * recip[None, :]).T
    return out, p
